# revision 19
# baseline (speedup 1.0000x reference)
"""EEND-SS loss device kernel (raw Bass, SPMD over 8 cores).

Device computes, per core (B_LOC=4 samples), the Gram matrix of
[sep rows(12) | src rows(12) | ones] over T=128000, chunk-blocked so the
host extracts all pairwise dots / sums / sq-sums for SI-SNR.
Host does the small O(B*T_sub) diarization BCE plus the tiny O(B)
permutation-invariant (PIT) assignment math and existence BCE.

Pipeline notes (HBM-bound; ~358 GB/s/core is the wall; 12.3 MB/core):
  - T remapped per phase so each (row, phase) slice is one contiguous HBM
    block: t = phase_off + p*W + n.  The Gram is a sum over all t, so any
    assignment of t -> (phase, partition, column) is valid.
  - Nonuniform phases W_LIST: big phases first (front-loads PE work),
    small last phase so the post-last-byte tail is short.
  - One dma_start per (tensor, phase): sync ring carries `sep`, scalar
    ring carries `src`; 1536 descriptors each (4*W bytes per descriptor,
    sequential HBM), so desc-gen (~3us) stays ahead of the ~8.7us/phase
    drain and the two rings split the HBM slots evenly.
  - No small-descriptor DMAs anywhere: a 96B-desc DMA steals a full
    packet slot per descriptor from the big stream (queues round-robin
    per packet), measured to cost ~4us of stream time.  That is why the
    diarization inputs stay on the host.

Blocked layout: free = (g, r, c): index = 130*g + 5*r + c, r in 0..25
 (rows 0..23 = data row r = t*12 + s*3 + i, row 24 = ones, row 25 = pad),
 c in 0..4, chunk n = 5*g + c.  Per phase, partition p holds T positions
 [off + p*W, off + (p+1)*W), chunk n is column n of that span.
Per block g one bf16 matmul, lhsT = rhs = blocked[:, 130g:130g+125]
  psum_gram[125,125] += lhsT.T @ rhs ; m = 5*r + c
  host: Gamma[ra, rb] = sum_c psum[5*ra+c, 5*rb+c]
"""

import numpy as np
from contextlib import ExitStack

import concourse.bass as bass
from concourse import mybir

F32 = mybir.dt.float32
BF16 = mybir.dt.bfloat16
AFT = mybir.ActivationFunctionType

C = 3
B_LOC = 4
P = 128
PD = 125
NROW = 26          # 24 data rows + ones(24) + pad(25)
NDATA = 24
BLK = 5 * NROW     # 130: free elems per chunk-group block
W_LIST = (250, 250, 250, 150, 100)  # columns per phase; each % 5 == 0
N_STAGE = 5        # staging slots (f32): one per phase, no WAR waits
R_SPLIT = 16       # last phase: DVE rows [0,16), ACT rows [16,24)


def build_nc(T=128000):
    """Build the per-core Bass program. Returns nc."""
    NPH = len(W_LIST)
    WMAX = max(W_LIST)
    GMAX = WMAX // 5
    assert sum(W_LIST) * P == T

    nc = bass.Bass(trn_type="TRN2", target_bir_lowering=False, debug=False)

    sep = nc.dram_tensor("sep", [B_LOC, C, T], F32, kind="ExternalInput").ap()
    src = nc.dram_tensor("src", [B_LOC, C, T], F32, kind="ExternalInput").ap()
    gram_out = nc.dram_tensor("gram", [PD, PD], F32, kind="ExternalOutput").ap()

    stg = [nc.alloc_sbuf_tensor(f"stg{i}", [P, NDATA * WMAX], F32).ap()
           for i in range(N_STAGE)]
    blk = [nc.alloc_sbuf_tensor(f"blk{i}", [P, NROW * WMAX], BF16).ap()
           for i in range(NPH)]
    out_sb = nc.alloc_sbuf_tensor("out_sb", [PD, PD], F32).ap()

    ps_g = nc.alloc_psum_tensor("ps_g", [PD, PD], F32).ap()

    def stg_dst(i):   # [p, t, sr, n] for DMA dst (row r = t*12 + s*3 + i)
        return stg[i].rearrange("p (t sr n) -> p t sr n", t=2, n=WMAX)

    def stg4(i):      # [p, r, g, c] source view for repack
        return stg[i].rearrange("p (r g c) -> p r g c", r=NDATA, g=GMAX)

    def blk4(i):      # [p, r, g, c] view of blocked (g, r, c) layout
        return blk[i].rearrange("p (g r c) -> p r g c", g=GMAX, r=NROW)

    off_list = []
    o = 0
    for w in W_LIST:
        off_list.append(o)
        o += P * w

    with ExitStack() as ctx:
        st_sems = [ctx.enter_context(nc.semaphore(f"st_sem{i}")) for i in range(NPH)]
        odma_sem = ctx.enter_context(nc.semaphore("odma_sem"))
        rpk_dve = ctx.enter_context(nc.semaphore("rpk_dve"))
        rpk_act = ctx.enter_context(nc.semaphore("rpk_act"))
        rpk_gp = ctx.enter_context(nc.semaphore("rpk_gp"))
        dve_sem = ctx.enter_context(nc.semaphore("dve_sem"))
        pe_sem = ctx.enter_context(nc.semaphore("pe_sem"))
        block = ctx.enter_context(nc.Block())

        def issue_phase_dmas(eng, ph, big, t):
            """One tensor's full phase slice in a single DMA (1536 descs)."""
            w = W_LIST[ph]
            sl = big[:, :, off_list[ph]:off_list[ph] + P * w].rearrange(
                "s i (p n) -> p (s i) n", p=P)
            d = stg_dst(ph)
            eng.dma_start(out=d[:, t, :, :w], in_=sl).then_inc(st_sems[ph], 16)

        @block.sync
        def _(sync: bass.BassEngine):
            for ph in range(NPH):
                issue_phase_dmas(sync, ph, sep, 0)
            sync.wait_ge(dve_sem, 1)
            sync.dma_start(out=gram_out, in_=out_sb).then_inc(odma_sem, 16)
            sync.wait_ge(odma_sem, 16)

        @block.gpsimd
        def _(gpsimd: bass.BassEngine):
            # ones(row 24) + pad(row 25) once per blocked slot
            for i in range(NPH):
                ap1 = blk[i].rearrange("p (g x) -> p g x", g=GMAX)[
                    :, :, 5 * NDATA:5 * NROW]
                gpsimd.memset(ap1, 1.0).then_inc(rpk_gp, 1)

        @block.scalar
        def _(scalar: bass.BassEngine):
            # pure DMA issuer: never let a repack wait starve the ring
            for ph in range(NPH):
                issue_phase_dmas(scalar, ph, src, 1)
            # last phase's tail rows, in parallel with DVE's head rows
            lp = NPH - 1
            scalar.wait_ge(st_sems[lp], 32)
            scalar.activation(
                blk4(lp)[:, R_SPLIT:NDATA, :W_LIST[lp] // 5, :],
                stg4(lp)[:, R_SPLIT:NDATA, :W_LIST[lp] // 5, :],
                AFT.Copy).then_inc(rpk_act, 1)

        @block.vector
        def _(vector: bass.BassEngine):
            for ph in range(NPH):
                rr1 = R_SPLIT if ph == NPH - 1 else NDATA
                vector.wait_ge(st_sems[ph], 32)
                vector.tensor_copy(
                    blk4(ph)[:, 0:rr1, :W_LIST[ph] // 5, :],
                    stg4(ph)[:, 0:rr1, :W_LIST[ph] // 5, :],
                ).then_inc(rpk_dve, 1)
            vector.wait_ge(pe_sem, NPH)
            vector.tensor_copy(out_sb, ps_g).then_inc(dve_sem, 1)

        @block.tensor
        def _(tensor: bass.BassEngine):
            nmm = 0
            total_mm = sum(w // 5 for w in W_LIST)
            for ph in range(NPH):
                tensor.wait_ge(rpk_dve, ph + 1)
                if ph == 0:
                    tensor.wait_ge(rpk_gp, NPH)
                if ph == NPH - 1:
                    tensor.wait_ge(rpk_act, 1)
                b = blk[ph]
                for g in range(W_LIST[ph] // 5):
                    ap = b[:, BLK * g: BLK * g + 125]
                    mm = tensor.matmul(ps_g, ap, ap,
                                       start=(nmm == 0), stop=(nmm == total_mm - 1))
                    nmm += 1
                mm.then_inc(pe_sem, 1)

    return nc


# ---------------- host side ----------------

EPS = 1e-8
LAM_SISNR, LAM_DIAR, LAM_EXIST = 1.0, 0.2, 0.2
TSUB = 1000
from itertools import permutations
PERMS = np.array(list(permutations(range(C))), dtype=np.int64)  # [6, 3]


def host_gamma(g125):
    """g125 [125,125] -> Gamma [25,25]; m = 5*r + c."""
    return np.einsum('acbc->ab', g125.reshape(25, 5, 25, 5).astype(np.float64))


def _clog(x):
    with np.errstate(divide='ignore'):
        return np.maximum(np.log(x), -100.0)


def host_diar_D(diar_probs, labels):
    """D[b,i,j] = mean_t BCE(pred[:,i], tgt_nn[:,j]) in float64."""
    T_f = labels.shape[1]
    idx = np.floor(np.arange(TSUB) * (T_f / TSUB)).astype(np.int64)
    tgt = labels[:, idx, :].astype(np.float64)             # [B, TSUB, C]
    p = diar_probs.astype(np.float64)
    logp = _clog(p)
    log1mp = _clog(1.0 - p)
    D = -(np.einsum('bti,btj->bij', logp, tgt)
          + np.einsum('bti,btj->bij', log1mp, 1.0 - tgt)) / TSUB
    return D


def host_finalize(gammas, D, exist_probs, num_speakers, T=128000):
    """gammas: list of [25,25] float64 per core; D [B,C,C] diar BCE matrix."""
    B = len(gammas) * B_LOC
    ns = np.asarray(num_speakers).astype(np.int64)

    S = np.zeros((B, C, C), np.float64)
    for core, gam in enumerate(gammas):
        for s in range(B_LOC):
            b = core * B_LOC + s
            e_rows = [s * 3 + i for i in range(3)]
            t_rows = [12 + s * 3 + j for j in range(3)]
            dot_raw = gam[np.ix_(e_rows, t_rows)]            # [i, j]
            sep_sq = np.array([gam[r, r] for r in e_rows])
            src_sq = np.array([gam[r, r] for r in t_rows])
            sum_sep = gam[e_rows, 24]
            sum_src = gam[t_rows, 24]

            dot = dot_raw - np.outer(sum_sep, sum_src) / T
            est_sq = sep_sq - sum_sep ** 2 / T               # [i]
            tgt_sq = src_sq - sum_src ** 2 / T               # [j]

            alpha = dot / (tgt_sq[None, :] + EPS)
            sig = alpha * alpha * tgt_sq[None, :] + EPS
            noise = est_sq[:, None] - 2.0 * alpha * dot + alpha * alpha * tgt_sq[None, :] + EPS
            S[b] = 10.0 * np.log10(sig / noise)

    n_spk = np.clip(ns, 1, C)
    slot = np.arange(C)
    slot_mask = (slot[None, :] < n_spk[:, None]).astype(np.float64)
    valid = np.all((PERMS[None, :, :] < n_spk[:, None, None])
                   | (slot[None, None, :] >= n_spk[:, None, None]), axis=-1)

    S_perm = S[:, PERMS, slot]                               # [B, 6, 3]
    sisnr_mean = (S_perm * slot_mask[:, None, :]).sum(-1) / n_spk[:, None]
    sisnr_loss_p = np.where(valid, -sisnr_mean, np.inf)
    best = sisnr_loss_p.min(axis=-1)
    loss_sisnr = best.mean()
    mean_sisnr = (-best).mean()

    D_perm = D[:, PERMS, slot]
    diar_p = (D_perm * slot_mask[:, None, :]).sum(-1) / n_spk[:, None]
    loss_diar = np.where(valid, diar_p, np.inf).min(axis=-1).mean()

    ep = np.asarray(exist_probs, np.float64)
    n_ex = np.minimum(ns, C)
    ex_tgt = (np.arange(C + 1)[None, :] < n_ex[:, None]).astype(np.float64)
    bce_ex = -(ex_tgt * _clog(ep) + (1.0 - ex_tgt) * _clog(1.0 - ep))
    loss_exist = bce_ex.mean()

    total = LAM_SISNR * loss_sisnr + LAM_DIAR * loss_diar + LAM_EXIST * loss_exist
    return tuple(np.float32(v) for v in
                 (total, loss_sisnr, loss_diar, loss_exist, mean_sisnr))


def shard_inputs(separated, diar_probs, sources, labels, n_cores=8):
    maps = []
    for c in range(n_cores):
        sl = slice(B_LOC * c, B_LOC * (c + 1))
        maps.append({
            "sep": np.ascontiguousarray(separated[sl], dtype=np.float32),
            "src": np.ascontiguousarray(sources[sl], dtype=np.float32),
        })
    return maps


# ---------------- kernel entry (self-contained) ----------------

N_CORES = 8
_CACHE = {}


def _get_nc():
    if "nc" not in _CACHE:
        _CACHE["nc"] = build_nc(T=128000)
    return _CACHE["nc"]


def kernel(separated, diar_probs, exist_probs, sources, labels, num_speakers):
    """EEND-SS loss on 8 NeuronCores: batch sharded 4 samples/core; device
    computes the big time-axis Grams; host does the small diar BCE and the
    tiny PIT/existence math."""
    from concourse.bass_utils import run_bass_kernel_spmd

    separated = np.asarray(separated)
    diar_probs = np.asarray(diar_probs)
    exist_probs = np.asarray(exist_probs)
    sources = np.asarray(sources)
    labels = np.asarray(labels)
    num_speakers = np.asarray(num_speakers)

    nc = _get_nc()
    in_maps = shard_inputs(separated, diar_probs, sources, labels, N_CORES)
    res = run_bass_kernel_spmd(nc, in_maps, list(range(N_CORES)))

    gammas = [host_gamma(res.results[c]["gram"]) for c in range(N_CORES)]
    D = host_diar_D(diar_probs, labels)
    return host_finalize(gammas, D, exist_probs, num_speakers, T=128000)


# revision 22
# speedup vs baseline: 1.0018x; 1.0018x over previous
"""EEND-SS loss device kernel (raw Bass, SPMD over 8 cores).

Device computes, per core (B_LOC=4 samples), the Gram matrix of
[sep rows(12) | src rows(12) | ones] over T=128000, chunk-blocked so the
host extracts all pairwise dots / sums / sq-sums for SI-SNR.
Host does the small O(B*T_sub) diarization BCE plus the tiny O(B)
permutation-invariant (PIT) assignment math and existence BCE.

Pipeline notes (HBM-bound; ~358 GB/s/core is the wall; 12.3 MB/core):
  - T remapped per phase so each (row, phase) slice is one contiguous HBM
    block: t = phase_off + p*W + n.  The Gram is a sum over all t, so any
    assignment of t -> (phase, partition, column) is valid.
  - Nonuniform phases W_LIST: big phases first (front-loads PE work),
    small last phase so the post-last-byte tail is short.
  - One dma_start per (tensor, phase): sync ring carries `sep`, scalar
    ring carries `src`; 1536 descriptors each (4*W bytes per descriptor,
    sequential HBM), so desc-gen (~3us) stays ahead of the ~8.7us/phase
    drain and the two rings split the HBM slots evenly.
  - No small-descriptor DMAs anywhere: a 96B-desc DMA steals a full
    packet slot per descriptor from the big stream (queues round-robin
    per packet), measured to cost ~4us of stream time.  That is why the
    diarization inputs stay on the host.

Blocked layout: free = (g, r, c): index = 130*g + 5*r + c, r in 0..25
 (rows 0..23 = data row r = t*12 + s*3 + i, row 24 = ones, row 25 = pad),
 c in 0..4, chunk n = 5*g + c.  Per phase, partition p holds T positions
 [off + p*W, off + (p+1)*W), chunk n is column n of that span.
Per block g one bf16 matmul, lhsT = rhs = blocked[:, 130g:130g+125]
  psum_gram[125,125] += lhsT.T @ rhs ; m = 5*r + c
  host: Gamma[ra, rb] = sum_c psum[5*ra+c, 5*rb+c]
"""

import numpy as np
from contextlib import ExitStack

import concourse.bass as bass
from concourse import mybir

F32 = mybir.dt.float32
BF16 = mybir.dt.bfloat16
AFT = mybir.ActivationFunctionType

C = 3
B_LOC = 4
P = 128
PD = 125
NROW = 26          # 24 data rows + ones(24) + pad(25)
NDATA = 24
BLK = 5 * NROW     # 130: free elems per chunk-group block
W_LIST = (250, 250, 250, 150, 100)  # columns per phase; each % 5 == 0
N_STAGE = 5        # staging slots (f32): one per phase, no WAR waits
R_SPLIT = 16       # last phase: DVE rows [0,16), ACT rows [16,24)


def build_nc(T=128000):
    """Build the per-core Bass program. Returns nc."""
    NPH = len(W_LIST)
    WMAX = max(W_LIST)
    GMAX = WMAX // 5
    assert sum(W_LIST) * P == T

    nc = bass.Bass(trn_type="TRN2", target_bir_lowering=False, debug=False)

    sep = nc.dram_tensor("sep", [B_LOC, C, T], F32, kind="ExternalInput").ap()
    src = nc.dram_tensor("src", [B_LOC, C, T], F32, kind="ExternalInput").ap()
    gram_out = nc.dram_tensor("gram", [PD, PD], F32, kind="ExternalOutput").ap()

    stg = [nc.alloc_sbuf_tensor(f"stg{i}", [P, NDATA * WMAX], F32).ap()
           for i in range(N_STAGE)]
    blk = [nc.alloc_sbuf_tensor(f"blk{i}", [P, NROW * WMAX], BF16).ap()
           for i in range(NPH)]
    out_sb = nc.alloc_sbuf_tensor("out_sb", [PD, PD], F32).ap()

    ps_g = nc.alloc_psum_tensor("ps_g", [PD, PD], F32).ap()

    def stg_dst(i):   # [p, t, sr, n] for DMA dst (row r = t*12 + s*3 + i)
        return stg[i].rearrange("p (t sr n) -> p t sr n", t=2, n=WMAX)

    def stg4(i):      # [p, r, g, c] source view for repack
        return stg[i].rearrange("p (r g c) -> p r g c", r=NDATA, g=GMAX)

    def blk4(i):      # [p, r, g, c] view of blocked (g, r, c) layout
        return blk[i].rearrange("p (g r c) -> p r g c", g=GMAX, r=NROW)

    off_list = []
    o = 0
    for w in W_LIST:
        off_list.append(o)
        o += P * w

    with ExitStack() as ctx:
        st_sems = [ctx.enter_context(nc.semaphore(f"st_sem{i}")) for i in range(NPH)]
        odma_sem = ctx.enter_context(nc.semaphore("odma_sem"))
        rpk_dve = ctx.enter_context(nc.semaphore("rpk_dve"))
        rpk_act = ctx.enter_context(nc.semaphore("rpk_act"))
        rpk_gp = ctx.enter_context(nc.semaphore("rpk_gp"))
        dve_sem = ctx.enter_context(nc.semaphore("dve_sem"))
        pe_sem = ctx.enter_context(nc.semaphore("pe_sem"))
        block = ctx.enter_context(nc.Block())

        def issue_phase_dmas(eng, ph, big, t, rows=(0, 12)):
            """One tensor's phase slice rows [rows) in a single DMA."""
            w = W_LIST[ph]
            sl = big[:, :, off_list[ph]:off_list[ph] + P * w].rearrange(
                "s i (p n) -> p (s i) n", p=P)
            d = stg_dst(ph)
            eng.dma_start(out=d[:, t, rows[0]:rows[1], :w],
                          in_=sl[:, rows[0]:rows[1], :],
                          ).then_inc(st_sems[ph], 16)

        # phase-0 is asymmetric: the scalar HWDGE queue starts draining ~2us
        # after the sync queue, so sync carries 15 of phase-0's 24 rows
        ST_TARGET = [48] + [32] * (NPH - 1)

        @block.sync
        def _(sync: bass.BassEngine):
            issue_phase_dmas(sync, 0, sep, 0)
            issue_phase_dmas(sync, 0, src, 1, rows=(0, 3))
            for ph in range(1, NPH):
                issue_phase_dmas(sync, ph, sep, 0)
            sync.wait_ge(dve_sem, 1)
            sync.dma_start(out=gram_out, in_=out_sb).then_inc(odma_sem, 16)
            sync.wait_ge(odma_sem, 16)

        @block.gpsimd
        def _(gpsimd: bass.BassEngine):
            # ones(row 24) + pad(row 25) once per blocked slot
            for i in range(NPH):
                ap1 = blk[i].rearrange("p (g x) -> p g x", g=GMAX)[
                    :, :, 5 * NDATA:5 * NROW]
                gpsimd.memset(ap1, 1.0).then_inc(rpk_gp, 1)

        @block.scalar
        def _(scalar: bass.BassEngine):
            # pure DMA issuer: never let a repack wait starve the ring
            issue_phase_dmas(scalar, 0, src, 1, rows=(3, 12))
            issue_phase_dmas(scalar, 1, src, 1)
            # warm the activation table while the ring drains
            scalar.activation(out_sb[0:1, 0:1], out_sb[0:1, 0:1], AFT.Copy)
            for ph in range(2, NPH):
                issue_phase_dmas(scalar, ph, src, 1)
            # last phase's tail rows, in parallel with DVE's head rows
            lp = NPH - 1
            scalar.wait_ge(st_sems[lp], ST_TARGET[lp])
            scalar.activation(
                blk4(lp)[:, R_SPLIT:NDATA, :W_LIST[lp] // 5, :],
                stg4(lp)[:, R_SPLIT:NDATA, :W_LIST[lp] // 5, :],
                AFT.Copy).then_inc(rpk_act, 1)

        @block.vector
        def _(vector: bass.BassEngine):
            for ph in range(NPH):
                rr1 = R_SPLIT if ph == NPH - 1 else NDATA
                vector.wait_ge(st_sems[ph], ST_TARGET[ph])
                vector.tensor_copy(
                    blk4(ph)[:, 0:rr1, :W_LIST[ph] // 5, :],
                    stg4(ph)[:, 0:rr1, :W_LIST[ph] // 5, :],
                ).then_inc(rpk_dve, 1)
            vector.wait_ge(pe_sem, NPH)
            vector.tensor_copy(out_sb, ps_g).then_inc(dve_sem, 1)

        @block.tensor
        def _(tensor: bass.BassEngine):
            nmm = 0
            total_mm = sum(w // 5 for w in W_LIST)
            for ph in range(NPH):
                tensor.wait_ge(rpk_dve, ph + 1)
                if ph == 0:
                    tensor.wait_ge(rpk_gp, NPH)
                if ph == NPH - 1:
                    tensor.wait_ge(rpk_act, 1)
                b = blk[ph]
                for g in range(W_LIST[ph] // 5):
                    ap = b[:, BLK * g: BLK * g + 125]
                    mm = tensor.matmul(ps_g, ap, ap,
                                       start=(nmm == 0), stop=(nmm == total_mm - 1))
                    nmm += 1
                mm.then_inc(pe_sem, 1)

    return nc


# ---------------- host side ----------------

EPS = 1e-8
LAM_SISNR, LAM_DIAR, LAM_EXIST = 1.0, 0.2, 0.2
TSUB = 1000
from itertools import permutations
PERMS = np.array(list(permutations(range(C))), dtype=np.int64)  # [6, 3]


def host_gamma(g125):
    """g125 [125,125] -> Gamma [25,25]; m = 5*r + c."""
    return np.einsum('acbc->ab', g125.reshape(25, 5, 25, 5).astype(np.float64))


def _clog(x):
    with np.errstate(divide='ignore'):
        return np.maximum(np.log(x), -100.0)


def host_diar_D(diar_probs, labels):
    """D[b,i,j] = mean_t BCE(pred[:,i], tgt_nn[:,j]) in float64."""
    T_f = labels.shape[1]
    idx = np.floor(np.arange(TSUB) * (T_f / TSUB)).astype(np.int64)
    tgt = labels[:, idx, :].astype(np.float64)             # [B, TSUB, C]
    p = diar_probs.astype(np.float64)
    logp = _clog(p)
    log1mp = _clog(1.0 - p)
    D = -(np.einsum('bti,btj->bij', logp, tgt)
          + np.einsum('bti,btj->bij', log1mp, 1.0 - tgt)) / TSUB
    return D


def host_finalize(gammas, D, exist_probs, num_speakers, T=128000):
    """gammas: list of [25,25] float64 per core; D [B,C,C] diar BCE matrix."""
    B = len(gammas) * B_LOC
    ns = np.asarray(num_speakers).astype(np.int64)

    S = np.zeros((B, C, C), np.float64)
    for core, gam in enumerate(gammas):
        for s in range(B_LOC):
            b = core * B_LOC + s
            e_rows = [s * 3 + i for i in range(3)]
            t_rows = [12 + s * 3 + j for j in range(3)]
            dot_raw = gam[np.ix_(e_rows, t_rows)]            # [i, j]
            sep_sq = np.array([gam[r, r] for r in e_rows])
            src_sq = np.array([gam[r, r] for r in t_rows])
            sum_sep = gam[e_rows, 24]
            sum_src = gam[t_rows, 24]

            dot = dot_raw - np.outer(sum_sep, sum_src) / T
            est_sq = sep_sq - sum_sep ** 2 / T               # [i]
            tgt_sq = src_sq - sum_src ** 2 / T               # [j]

            alpha = dot / (tgt_sq[None, :] + EPS)
            sig = alpha * alpha * tgt_sq[None, :] + EPS
            noise = est_sq[:, None] - 2.0 * alpha * dot + alpha * alpha * tgt_sq[None, :] + EPS
            S[b] = 10.0 * np.log10(sig / noise)

    n_spk = np.clip(ns, 1, C)
    slot = np.arange(C)
    slot_mask = (slot[None, :] < n_spk[:, None]).astype(np.float64)
    valid = np.all((PERMS[None, :, :] < n_spk[:, None, None])
                   | (slot[None, None, :] >= n_spk[:, None, None]), axis=-1)

    S_perm = S[:, PERMS, slot]                               # [B, 6, 3]
    sisnr_mean = (S_perm * slot_mask[:, None, :]).sum(-1) / n_spk[:, None]
    sisnr_loss_p = np.where(valid, -sisnr_mean, np.inf)
    best = sisnr_loss_p.min(axis=-1)
    loss_sisnr = best.mean()
    mean_sisnr = (-best).mean()

    D_perm = D[:, PERMS, slot]
    diar_p = (D_perm * slot_mask[:, None, :]).sum(-1) / n_spk[:, None]
    loss_diar = np.where(valid, diar_p, np.inf).min(axis=-1).mean()

    ep = np.asarray(exist_probs, np.float64)
    n_ex = np.minimum(ns, C)
    ex_tgt = (np.arange(C + 1)[None, :] < n_ex[:, None]).astype(np.float64)
    bce_ex = -(ex_tgt * _clog(ep) + (1.0 - ex_tgt) * _clog(1.0 - ep))
    loss_exist = bce_ex.mean()

    total = LAM_SISNR * loss_sisnr + LAM_DIAR * loss_diar + LAM_EXIST * loss_exist
    return tuple(np.float32(v) for v in
                 (total, loss_sisnr, loss_diar, loss_exist, mean_sisnr))


def shard_inputs(separated, diar_probs, sources, labels, n_cores=8):
    maps = []
    for c in range(n_cores):
        sl = slice(B_LOC * c, B_LOC * (c + 1))
        maps.append({
            "sep": np.ascontiguousarray(separated[sl], dtype=np.float32),
            "src": np.ascontiguousarray(sources[sl], dtype=np.float32),
        })
    return maps


# ---------------- kernel entry (self-contained) ----------------

N_CORES = 8
_CACHE = {}


def _get_nc():
    if "nc" not in _CACHE:
        _CACHE["nc"] = build_nc(T=128000)
    return _CACHE["nc"]


def kernel(separated, diar_probs, exist_probs, sources, labels, num_speakers):
    """EEND-SS loss on 8 NeuronCores: batch sharded 4 samples/core; device
    computes the big time-axis Grams; host does the small diar BCE and the
    tiny PIT/existence math."""
    from concourse.bass_utils import run_bass_kernel_spmd

    separated = np.asarray(separated)
    diar_probs = np.asarray(diar_probs)
    exist_probs = np.asarray(exist_probs)
    sources = np.asarray(sources)
    labels = np.asarray(labels)
    num_speakers = np.asarray(num_speakers)

    nc = _get_nc()
    in_maps = shard_inputs(separated, diar_probs, sources, labels, N_CORES)
    res = run_bass_kernel_spmd(nc, in_maps, list(range(N_CORES)))

    gammas = [host_gamma(res.results[c]["gram"]) for c in range(N_CORES)]
    D = host_diar_D(diar_probs, labels)
    return host_finalize(gammas, D, exist_probs, num_speakers, T=128000)


# revision 31
# speedup vs baseline: 1.0316x; 1.0298x over previous
"""EEND-SS loss device kernel (raw Bass, SPMD over 8 cores).

Device computes, per core (B_LOC=4 samples), the Gram matrix of
[sep rows(12) | src rows(12) | ones] over T=128000, chunk-blocked so the
host extracts all pairwise dots / sums / sq-sums for SI-SNR.
Host does the small O(B*T_sub) diarization BCE plus the tiny O(B)
permutation-invariant (PIT) assignment math and existence BCE.

Pipeline notes (HBM-bound; ~358 GB/s/core is the wall; 12.3 MB/core):
  - T remapped per phase so each (row, phase) slice is one contiguous HBM
    block: t = phase_off + p*W + n.  The Gram is a sum over all t, so any
    assignment of t -> (phase, partition, column) is valid.
  - Nonuniform phases W_LIST: big phases first (front-loads PE work),
    small last phase so the post-last-byte tail is short.
  - One dma_start per (tensor, phase): sync ring carries `sep`, scalar
    ring carries `src`; 1536 descriptors each (4*W bytes per descriptor,
    sequential HBM), so desc-gen (~3us) stays ahead of the ~8.7us/phase
    drain and the two rings split the HBM slots evenly.
  - No small-descriptor DMAs anywhere: a 96B-desc DMA steals a full
    packet slot per descriptor from the big stream (queues round-robin
    per packet), measured to cost ~4us of stream time.  That is why the
    diarization inputs stay on the host.

Blocked layout: free = (g, r, c): index = 130*g + 5*r + c, r in 0..25
 (rows 0..23 = data row r = t*12 + s*3 + i, row 24 = ones, row 25 = pad),
 c in 0..4, chunk n = 5*g + c.  Per phase, partition p holds T positions
 [off + p*W, off + (p+1)*W), chunk n is column n of that span.
Per block g one bf16 matmul, lhsT = rhs = blocked[:, 130g:130g+125]
  psum_gram[125,125] += lhsT.T @ rhs ; m = 5*r + c
  host: Gamma[ra, rb] = sum_c psum[5*ra+c, 5*rb+c]
"""

import numpy as np
from contextlib import ExitStack

import concourse.bass as bass
from concourse import mybir

F32 = mybir.dt.float32
BF16 = mybir.dt.bfloat16
AFT = mybir.ActivationFunctionType

C = 3
B_LOC = 4
P = 128
PD = 125
NROW = 26          # 24 data rows + ones(24) + pad(25)
NDATA = 24
BLK = 5 * NROW     # 130: free elems per chunk-group block
W_LIST = (250, 250, 250, 150, 100)  # columns per phase; each % 5 == 0
N_STAGE = 5        # staging slots (f32): one per phase, no WAR waits
R_SPLIT = 16       # last phase: DVE rows [0,16), ACT rows [16,24)


def build_nc(T=128000):
    """Build the per-core Bass program. Returns nc."""
    NPH = len(W_LIST)
    WMAX = max(W_LIST)
    GMAX = WMAX // 5
    assert sum(W_LIST) * P == T

    nc = bass.Bass(trn_type="TRN2", target_bir_lowering=False, debug=False)

    sep = nc.dram_tensor("sep", [B_LOC, C, T], F32, kind="ExternalInput").ap()
    src = nc.dram_tensor("src", [B_LOC, C, T], F32, kind="ExternalInput").ap()
    gram_out = nc.dram_tensor("gram", [PD, PD], F32, kind="ExternalOutput").ap()

    stg = [nc.alloc_sbuf_tensor(f"stg{i}", [P, NDATA * WMAX], F32).ap()
           for i in range(N_STAGE)]
    blk = [nc.alloc_sbuf_tensor(f"blk{i}", [P, NROW * WMAX], BF16).ap()
           for i in range(NPH)]
    out_sb = nc.alloc_sbuf_tensor("out_sb", [PD, PD], F32).ap()

    ps_g = nc.alloc_psum_tensor("ps_g", [PD, PD], F32).ap()

    def stg_dst(i):   # [p, t, sr, n] for DMA dst (row r = t*12 + s*3 + i)
        return stg[i].rearrange("p (t sr n) -> p t sr n", t=2, n=WMAX)

    def stg4(i):      # [p, r, g, c] source view for repack
        return stg[i].rearrange("p (r g c) -> p r g c", r=NDATA, g=GMAX)

    def blk4(i):      # [p, r, g, c] view of blocked (g, r, c) layout
        return blk[i].rearrange("p (g r c) -> p r g c", g=GMAX, r=NROW)

    off_list = []
    o = 0
    for w in W_LIST:
        off_list.append(o)
        o += P * w

    with ExitStack() as ctx:
        st_sems = [ctx.enter_context(nc.semaphore(f"st_sem{i}")) for i in range(NPH)]
        odma_sem = ctx.enter_context(nc.semaphore("odma_sem"))
        rpk_dve = ctx.enter_context(nc.semaphore("rpk_dve"))
        rpk_act = ctx.enter_context(nc.semaphore("rpk_act"))
        rpk_gp = ctx.enter_context(nc.semaphore("rpk_gp"))
        warm_sem = ctx.enter_context(nc.semaphore("warm_sem"))
        dve_sem = ctx.enter_context(nc.semaphore("dve_sem"))
        pe_sem = ctx.enter_context(nc.semaphore("pe_sem"))
        block = ctx.enter_context(nc.Block())

        def issue_phase_dmas(eng, ph, big, t, rows=(0, 12)):
            """One tensor's phase slice rows [rows) in a single DMA."""
            w = W_LIST[ph]
            sl = big[:, :, off_list[ph]:off_list[ph] + P * w].rearrange(
                "s i (p n) -> p (s i) n", p=P)
            d = stg_dst(ph)
            eng.dma_start(out=d[:, t, rows[0]:rows[1], :w],
                          in_=sl[:, rows[0]:rows[1], :],
                          ).then_inc(st_sems[ph], 16)

        # the scalar HWDGE queue starts draining ~3.5us after sync's and the
        # queues then split slots evenly, so hand sync ~0.26MB of scalar's
        # work (one src row in each of phases 0/1) to equalize finish times
        ST_TARGET = [48, 48] + [32] * (NPH - 2)

        @block.sync
        def _(sync: bass.BassEngine):
            for ph in range(NPH):
                issue_phase_dmas(sync, ph, sep, 0)
                if ph < 2:
                    issue_phase_dmas(sync, ph, src, 1, rows=(0, 1))
            # output split across both rings so the HBM write receipts overlap
            sync.wait_ge(dve_sem, 1)
            sync.dma_start(out=gram_out[0:63, :],
                           in_=out_sb[0:63, :]).then_inc(odma_sem, 16)
            sync.wait_ge(odma_sem, 32)

        @block.gpsimd
        def _(gpsimd: bass.BassEngine):
            # ones(row 24) + pad(row 25) once per blocked slot
            for i in range(NPH):
                ap1 = blk[i].rearrange("p (g x) -> p g x", g=GMAX)[
                    :, :, 5 * NDATA:5 * NROW]
                gpsimd.memset(ap1, 1.0).then_inc(rpk_gp, 1)

        @block.scalar
        def _(scalar: bass.BassEngine):
            # 1-desc dummy first: arms the scalar HWDGE queue, whose first
            # real DMA otherwise starts draining ~2us after sync's
            scalar.dma_start(out=out_sb[0:1, 0:64], in_=sep[0, 0, 0:64]
                             ).then_inc(warm_sem, 16)
            # pure DMA issuer: never let a repack wait starve the ring
            issue_phase_dmas(scalar, 0, src, 1, rows=(1, 12))
            issue_phase_dmas(scalar, 1, src, 1, rows=(1, 12))
            # warm the activation table while the ring drains
            scalar.activation(out_sb[0:1, 0:1], out_sb[0:1, 0:1], AFT.Copy)
            for ph in range(2, NPH):
                issue_phase_dmas(scalar, ph, src, 1)
            # last phase, second half of chunk-groups (DVE does the first)
            lp = NPH - 1
            gh = (W_LIST[lp] // 5) // 2
            scalar.wait_ge(st_sems[lp], ST_TARGET[lp])
            scalar.activation(
                blk4(lp)[:, 0:NDATA, gh:2 * gh, :],
                stg4(lp)[:, 0:NDATA, gh:2 * gh, :],
                AFT.Copy).then_inc(rpk_act, 1)
            # second half of the output write
            scalar.wait_ge(dve_sem, 1)
            scalar.dma_start(out=gram_out[63:PD, :],
                             in_=out_sb[63:PD, :]).then_inc(odma_sem, 16)

        @block.vector
        def _(vector: bass.BassEngine):
            for ph in range(NPH):
                gl = W_LIST[ph] // 5
                if ph == NPH - 1:
                    gl = gl // 2      # last phase: first half; ACT does the rest
                vector.wait_ge(st_sems[ph], ST_TARGET[ph])
                vector.tensor_copy(
                    blk4(ph)[:, 0:NDATA, :gl, :],
                    stg4(ph)[:, 0:NDATA, :gl, :],
                ).then_inc(rpk_dve, 1)
            vector.wait_ge(pe_sem, NPH)
            vector.tensor_copy(out_sb, ps_g).then_inc(dve_sem, 1)

        @block.tensor
        def _(tensor: bass.BassEngine):
            nmm = 0
            total_mm = sum(w // 5 for w in W_LIST)
            for ph in range(NPH):
                tensor.wait_ge(rpk_dve, ph + 1)
                if ph == 0:
                    tensor.wait_ge(rpk_gp, NPH)
                b = blk[ph]
                gl = W_LIST[ph] // 5
                for g in range(gl):
                    if ph == NPH - 1 and g == gl // 2:
                        tensor.wait_ge(rpk_act, 1)
                    ap = b[:, BLK * g: BLK * g + 125]
                    mm = tensor.matmul(ps_g, ap, ap,
                                       start=(nmm == 0), stop=(nmm == total_mm - 1))
                    nmm += 1
                mm.then_inc(pe_sem, 1)

    return nc


# ---------------- host side ----------------

EPS = 1e-8
LAM_SISNR, LAM_DIAR, LAM_EXIST = 1.0, 0.2, 0.2
TSUB = 1000
from itertools import permutations
PERMS = np.array(list(permutations(range(C))), dtype=np.int64)  # [6, 3]


def host_gamma(g125):
    """g125 [125,125] -> Gamma [25,25]; m = 5*r + c."""
    return np.einsum('acbc->ab', g125.reshape(25, 5, 25, 5).astype(np.float64))


def _clog(x):
    with np.errstate(divide='ignore'):
        return np.maximum(np.log(x), -100.0)


def host_diar_D(diar_probs, labels):
    """D[b,i,j] = mean_t BCE(pred[:,i], tgt_nn[:,j]) in float64."""
    T_f = labels.shape[1]
    idx = np.floor(np.arange(TSUB) * (T_f / TSUB)).astype(np.int64)
    tgt = labels[:, idx, :].astype(np.float64)             # [B, TSUB, C]
    p = diar_probs.astype(np.float64)
    logp = _clog(p)
    log1mp = _clog(1.0 - p)
    D = -(np.einsum('bti,btj->bij', logp, tgt)
          + np.einsum('bti,btj->bij', log1mp, 1.0 - tgt)) / TSUB
    return D


def host_finalize(gammas, D, exist_probs, num_speakers, T=128000):
    """gammas: list of [25,25] float64 per core; D [B,C,C] diar BCE matrix."""
    B = len(gammas) * B_LOC
    ns = np.asarray(num_speakers).astype(np.int64)

    S = np.zeros((B, C, C), np.float64)
    for core, gam in enumerate(gammas):
        for s in range(B_LOC):
            b = core * B_LOC + s
            e_rows = [s * 3 + i for i in range(3)]
            t_rows = [12 + s * 3 + j for j in range(3)]
            dot_raw = gam[np.ix_(e_rows, t_rows)]            # [i, j]
            sep_sq = np.array([gam[r, r] for r in e_rows])
            src_sq = np.array([gam[r, r] for r in t_rows])
            sum_sep = gam[e_rows, 24]
            sum_src = gam[t_rows, 24]

            dot = dot_raw - np.outer(sum_sep, sum_src) / T
            est_sq = sep_sq - sum_sep ** 2 / T               # [i]
            tgt_sq = src_sq - sum_src ** 2 / T               # [j]

            alpha = dot / (tgt_sq[None, :] + EPS)
            sig = alpha * alpha * tgt_sq[None, :] + EPS
            noise = est_sq[:, None] - 2.0 * alpha * dot + alpha * alpha * tgt_sq[None, :] + EPS
            S[b] = 10.0 * np.log10(sig / noise)

    n_spk = np.clip(ns, 1, C)
    slot = np.arange(C)
    slot_mask = (slot[None, :] < n_spk[:, None]).astype(np.float64)
    valid = np.all((PERMS[None, :, :] < n_spk[:, None, None])
                   | (slot[None, None, :] >= n_spk[:, None, None]), axis=-1)

    S_perm = S[:, PERMS, slot]                               # [B, 6, 3]
    sisnr_mean = (S_perm * slot_mask[:, None, :]).sum(-1) / n_spk[:, None]
    sisnr_loss_p = np.where(valid, -sisnr_mean, np.inf)
    best = sisnr_loss_p.min(axis=-1)
    loss_sisnr = best.mean()
    mean_sisnr = (-best).mean()

    D_perm = D[:, PERMS, slot]
    diar_p = (D_perm * slot_mask[:, None, :]).sum(-1) / n_spk[:, None]
    loss_diar = np.where(valid, diar_p, np.inf).min(axis=-1).mean()

    ep = np.asarray(exist_probs, np.float64)
    n_ex = np.minimum(ns, C)
    ex_tgt = (np.arange(C + 1)[None, :] < n_ex[:, None]).astype(np.float64)
    bce_ex = -(ex_tgt * _clog(ep) + (1.0 - ex_tgt) * _clog(1.0 - ep))
    loss_exist = bce_ex.mean()

    total = LAM_SISNR * loss_sisnr + LAM_DIAR * loss_diar + LAM_EXIST * loss_exist
    return tuple(np.float32(v) for v in
                 (total, loss_sisnr, loss_diar, loss_exist, mean_sisnr))


def shard_inputs(separated, diar_probs, sources, labels, n_cores=8):
    maps = []
    for c in range(n_cores):
        sl = slice(B_LOC * c, B_LOC * (c + 1))
        maps.append({
            "sep": np.ascontiguousarray(separated[sl], dtype=np.float32),
            "src": np.ascontiguousarray(sources[sl], dtype=np.float32),
        })
    return maps


# ---------------- kernel entry (self-contained) ----------------

N_CORES = 8
_CACHE = {}


def _get_nc():
    if "nc" not in _CACHE:
        _CACHE["nc"] = build_nc(T=128000)
    return _CACHE["nc"]


def kernel(separated, diar_probs, exist_probs, sources, labels, num_speakers):
    """EEND-SS loss on 8 NeuronCores: batch sharded 4 samples/core; device
    computes the big time-axis Grams; host does the small diar BCE and the
    tiny PIT/existence math."""
    from concourse.bass_utils import run_bass_kernel_spmd

    separated = np.asarray(separated)
    diar_probs = np.asarray(diar_probs)
    exist_probs = np.asarray(exist_probs)
    sources = np.asarray(sources)
    labels = np.asarray(labels)
    num_speakers = np.asarray(num_speakers)

    nc = _get_nc()
    in_maps = shard_inputs(separated, diar_probs, sources, labels, N_CORES)
    res = run_bass_kernel_spmd(nc, in_maps, list(range(N_CORES)))

    gammas = [host_gamma(res.results[c]["gram"]) for c in range(N_CORES)]
    D = host_diar_D(diar_probs, labels)
    return host_finalize(gammas, D, exist_probs, num_speakers, T=128000)
